# revision 28
# baseline (speedup 1.0000x reference)
"""Trainium2 Bass kernel for CapsNet dynamic routing (nn_Model_16492674417055).

Reference computation:
    u_hat[b,i,j,c,p] = sum_q w[j,c,p,q] x[b,i,c,q]
    3 routing iterations of: c = softmax_j(b); s = sum_i c*u_hat;
    v = squash(s); a = <u_hat, v>; b += a. Output v of last iteration.

Key algebraic factorization (exact in real arithmetic): u_hat never needs to
be materialized (it is 1 GiB).  With xc[b,j,c,:] = sum_i c[b,i,j,c] x[b,i,c,:]:
    s  = W @ xc
    a  = <x_i, W^T v>  and  W^T v = kappa * (W^T W) xc = kappa * G xc,
where kappa is the squash scale, computable from |s|^2 = <xc, G xc>.
So iterations 1..2 need only G = W^T W (host-precomputed), and the final
iteration needs one true W application for the output direction.

Sharding: data-parallel over batch B=16 across 8 cores (2 batches/core);
G / wT are replicated (loaded per core); routing state stays core-local.

Design notes (v3):
- Matmul operands in bf16 (DT_X for x/c/gx, DT_W for G/wt/xc): fp32 matmuls
  cost 4 cycles/row and serialize the weight load; bf16 runs 1 cycle/row with
  fast-weight-load LDWEIGHTS that overlap the previous matmul.  All routing
  state stays fp32 (PSUM accumulation, logits, softmax, squash scalars).
  Measured rel err 1.6e-2 vs the 2e-2 gate (fp32 everywhere gives 3e-6).
- kappa is folded into the logit update (bl += kappa * <x_i, gx>) instead of
  scaling vt = kappa*gx before the A-pass, so the 64 A-matmuls start right
  after the W-pass while the squash scalar chain runs in parallel.
- xk/g/xt are double-buffered: in the benchmark loop each body prefetches the
  next body's buffers at its start, so input DMA overlaps the previous body's
  tail instead of stalling each rep's head.  wt is single-buffered (read late
  at t=2, loaded early per body).
- XC accumulates all 4 channels of one batch into a single PSUM bank
  (32 matmuls), one PSUM->SBUF cast-copy per batch.
- tc.For_i runs an all-engine barrier + semaphore reset every loop iteration,
  which drains the cross-body pipeline; the bench loop therefore unrolls U=4
  bodies per For_i iteration (113us/rep at U=2 -> ~76-91us at U=4 under the
  paired-slope protocol; U=8 was worse).
- PSUM tiles must be released by SHORT-queue engines: every PSUM consumer
  whose result feeds DVE reads an ACT-made SBUF copy (gx_bf, sq_sb, kb_sb).
  A DVE read of PSUM parks the bank behind the deep DVE queue and stalls the
  PE matmuls that want the bank two iterations later (measured 2x slowdown).
- Roofline context: ~704 LDW+MM pairs x ~55-80ns ~= 39-56us on PE and 12 MiB
  of input DMA ~= 35-55us per core; measured per-rep sits just above both.
"""

import numpy as np

import concourse.bass as bass
import concourse.tile as tile
from concourse import bacc
from concourse import mybir
from concourse.alu_op_type import AluOpType as AO
from concourse.bass import MemorySpace
from concourse.bass_utils import run_bass_kernel_spmd
from concourse.masks import make_identity

F32 = mybir.dt.float32
BF16 = mybir.dt.bfloat16
AXX = mybir.AxisListType.X
AF = mybir.ActivationFunctionType

N_CORES = 8
B, N_PRE, N_DIGIT, CH, D = 16, 1024, 32, 4, 128
BL = B // N_CORES          # batches per core (2)
NCHUNK = N_PRE // 128      # i-chunks (8)
NJC = N_DIGIT * CH         # 128 (j,c) pairs
EPS = 1e-7
N_ITERS = 3

# matmul operand dtypes (state is always fp32); matmul needs matching operand
# dtypes: DT_X covers xk/xt/softmax-c/gx(A-pass rhs), DT_W covers G/wT/xc.
DT_X = BF16
DT_W = BF16


class _Bacc(bacc.Bacc):
    """Bacc whose ACT-table chooser only sees natural_log_exp_and_others, so
    alternating Exp (softmax) / Ln+Exp (squash sqrt) stay on ONE table set
    (one LoadActFuncSet instead of one per switch)."""

    def insert_act_table_loads(self):
        from concourse.hw_specs import get_activation_tables

        has_activation = any(
            isinstance(i, mybir.InstActivation)
            for b in self.main_func.blocks
            for i in b.instructions
        )
        if not has_activation:
            return
        tables = [
            (n, fns if n == "natural_log_exp_and_others" else set())
            for n, fns in get_activation_tables(self.m.arch).items()
        ]
        bacc._bass_rust.insert_act_table_loads(self, tables)


def build_nc(bench_reps: int = 0, bench_mode: str = "full") -> bass.Bass:
    """bench_reps>0 wraps the kernel body (input DMAs included) in a For_i
    loop (two ping-pong bodies per iteration) inside one NEFF, for wall-clock
    timing that amortizes the multi-ms axon dispatch floor."""
    nc = _Bacc()

    # Per-core DRAM inputs, host pre-laid-out so every load is a straight
    # [128, N] partition-major copy.
    xk_d = nc.declare_dram_parameter("xk", [128, BL, CH, NCHUNK, 128], DT_X, isOutput=False)  # [i128, b, c, k, q]
    xt_d = nc.declare_dram_parameter("xt", [128, BL, CH, NCHUNK, 128], DT_X, isOutput=False)  # [q, b, c, k, i128]
    g_d = nc.declare_dram_parameter("g", [128, NJC, 128], DT_W, isOutput=False)               # [q, (c j), q']
    wt_d = nc.declare_dram_parameter("wt", [128, NJC, 128], DT_W, isOutput=False)             # [q, (c j), p]
    out_d = nc.declare_dram_parameter("out", [BL, N_DIGIT, CH, D], F32, isOutput=True)

    with tile.TileContext(nc) as tc:
        with (
            tc.tile_pool(name="big", bufs=1) as big,
            tc.tile_pool(name="sm", bufs=2) as sm,
            tc.tile_pool(name="ps_xc", bufs=2, space=MemorySpace.PSUM) as ps_xc,
            tc.tile_pool(name="ps_gx", bufs=2, space=MemorySpace.PSUM) as ps_gx,
            tc.tile_pool(name="ps_a", bufs=2, space=MemorySpace.PSUM) as ps_a,
            tc.tile_pool(name="ps_sq", bufs=1, space=MemorySpace.PSUM) as ps_sq,
            tc.tile_pool(name="ps_kb", bufs=1, space=MemorySpace.PSUM) as ps_kb,
        ):
            # ---- static tiles (xk/g/xt ping-pong buffered) ----
            xk2 = [big.tile([128, BL, CH, NCHUNK, 128], DT_X, tag=f"xk{p}", name=f"xk{p}") for p in range(2)]
            xt2 = [big.tile([128, BL, CH, NCHUNK, 128], DT_X, tag=f"xt{p}", name=f"xt{p}") for p in range(2)]
            g2 = [big.tile([128, NJC, 128], DT_W, tag=f"g{p}", name=f"g{p}") for p in range(2)]
            wt = big.tile([128, NJC, 128], DT_W, tag="wt")

            c_unif = big.tile([128, N_DIGIT], DT_X, tag="c_unif")
            nc.vector.memset(c_unif, 1.0 / N_DIGIT)
            ones_col = big.tile([128, 1], F32, tag="ones_col")
            nc.vector.memset(ones_col, 1.0)
            ones_row = big.tile([1, 128], F32, tag="ones_row")
            nc.vector.memset(ones_row, 1.0)
            ident = big.tile([128, 128], F32, tag="ident")
            make_identity(nc, ident[:])
            eps_t = big.tile([1, 1], F32, tag="eps_t")
            nc.vector.memset(eps_t, EPS)

            # routing logits per local batch: [i%128, k, c, j]
            bl_t = [big.tile([128, NCHUNK, CH, N_DIGIT], F32, tag=f"bl{b}", name=f"bl{b}") for b in range(BL)]

            def load_x_g(p):
                nc.sync.dma_start(out=xk2[p][:], in_=xk_d[:])
                nc.scalar.dma_start(out=g2[p][:], in_=g_d[:])
                nc.scalar.dma_start(out=xt2[p][:], in_=xt_d[:])

            def load_wt():
                nc.sync.dma_start(out=wt[:], in_=wt_d[:])

            def trace_body(p, prefetch=False, loads=True, compute=True):
              # prefetch=True (pipelined bench loop): load NEXT parity's
              # x/g buffers + this body's wt.  Otherwise self-load parity p.
              if prefetch:
                  load_x_g(1 - p)
                  load_wt()
              elif loads:
                  load_x_g(p)
                  load_wt()
              if not compute:
                  return
              xk, xt, gt = xk2[p], xt2[p], g2[p]
              for t in range(N_ITERS):
                  last = t == N_ITERS - 1

                  # ---- softmax over j (t=0: uniform, skip) ----
                  cbt = []
                  if t > 0:
                      for b in range(BL):
                          # b=0 on DVE, b=1 on GpSimd (parallel chains)
                          ve = nc.vector if b == 0 else nc.gpsimd
                          blv = bl_t[b][:]
                          mx = sm.tile([128, NCHUNK, CH], F32, tag=f"mx{b}")
                          nc.vector.reduce_max(out=mx[:], in_=blv, axis=AXX, negate=True)
                          eb = sm.tile([128, NCHUNK, CH, N_DIGIT], DT_X, tag=f"e{b}")
                          ve.tensor_add(eb[:], blv, mx[:].to_broadcast(eb.shape))
                          nc.scalar.activation(eb[:], eb[:], AF.Exp)
                          sb = sm.tile([128, NCHUNK, CH], F32, tag=f"sum{b}")
                          nc.vector.reduce_sum(out=sb[:], in_=eb[:], axis=AXX)
                          nc.vector.reciprocal(sb[:], sb[:])
                          # normalize in place: eb becomes the c coefficients
                          ve.tensor_mul(eb[:], eb[:], sb[:].to_broadcast(eb.shape))
                          cbt.append(eb)

                  # ---- XC: xcT[q, (c,j,b)]; 4 channels share one PSUM bank ----
                  xc_sb = sm.tile([128, CH, N_DIGIT, BL], DT_W, tag="xc", bufs=3)
                  for b in range(BL):
                      xc_ps = ps_xc.tile([128, CH, N_DIGIT], F32, tag="xc_ps")
                      for c in range(CH):
                          for k in range(NCHUNK):
                              rhs = cbt[b][:, k, c, :] if t > 0 else c_unif[:]
                              nc.tensor.matmul(
                                  xc_ps[:, c, :],
                                  lhsT=xk[:, b, c, k, :],
                                  rhs=rhs,
                                  start=(k == 0),
                                  stop=(k == NCHUNK - 1),
                              )
                      nc.vector.tensor_copy(xc_sb[:, :, :, b], xc_ps[:])

                  # ---- W-pass: gxT = G @ xc (t<2)  /  sT = W @ xc (t=2) ----
                  gx_ps = ps_gx.tile([128, CH, N_DIGIT, BL], F32, tag="gx")
                  wsrc = wt if last else gt
                  for jc in range(NJC):
                      c, j = divmod(jc, N_DIGIT)
                      nc.tensor.matmul(
                          gx_ps[:, c, j, :],
                          lhsT=wsrc[:, jc, :],
                          rhs=xc_sb[:, c, j, :],
                          start=True,
                          stop=True,
                      )

                  # ---- squash scale kappa (batched over c,j,b) ----
                  # |s|^2 = <xc, G xc> (t<2) or <s, s> (t=2);
                  # kappa = sq/((1+sq)*sqrt(sq+eps)); sqrt = exp(0.5*ln) so only
                  # the natural_log_exp ACT table set is used.
                  xg = sm.tile([128, CH, N_DIGIT, BL], F32, tag="xg")
                  sq_ps = ps_sq.tile([1, CH * N_DIGIT * BL], F32, tag="sq")
                  kb_ps = ps_kb.tile([128, CH, N_DIGIT, BL], F32, tag="kb")
                  t1 = sm.tile([1, CH * N_DIGIT * BL], F32, tag="t1")
                  t2 = sm.tile([1, CH * N_DIGIT * BL], F32, tag="t2")
                  kap = sm.tile([1, CH * N_DIGIT * BL], F32, tag="kap")
                  if not last:
                      # bf16 copy of gx feeds the A-pass immediately; kappa is
                      # applied later, during the logit update.  xg reads the
                      # SBUF copy, not gx_ps: the PSUM bank must be released by
                      # the short ACT queue, not the deep DVE queue, or the
                      # W-pass two iterations later stalls on the rotation.
                      gx_bf = sm.tile([128, CH, N_DIGIT, BL], DT_X, tag="gx_bf", bufs=3)
                      nc.scalar.copy(out=gx_bf[:], in_=gx_ps[:])
                      nc.vector.tensor_mul(xg[:], xc_sb[:], gx_bf[:])
                  else:
                      gx_f = sm.tile([128, CH, N_DIGIT, BL], F32, tag="gx_f")
                      nc.scalar.copy(out=gx_f[:], in_=gx_ps[:])
                      nc.vector.tensor_mul(xg[:], gx_f[:], gx_f[:])
                  nc.tensor.matmul(
                      sq_ps[:],
                      lhsT=ones_col[:],
                      rhs=xg[:].rearrange("p a b c -> p (a b c)"),
                      start=True,
                      stop=True,
                  )
                  # sq leaves PSUM via the short ACT queue (releases the bank
                  # for the next iteration's sq matmul); DVE reads the copy.
                  sq_sb = sm.tile([1, CH * N_DIGIT * BL], F32, tag="sq_sb")
                  nc.scalar.copy(out=sq_sb[:], in_=sq_ps[:])
                  nc.scalar.activation(t1[:], sq_sb[:], AF.Ln, bias=eps_t[:])
                  nc.scalar.activation(t1[:], t1[:], AF.Exp, scale=0.5)
                  nc.vector.scalar_tensor_tensor(
                      out=t2[:], in0=sq_sb[:], scalar=1.0,
                      in1=t1[:], op0=AO.add, op1=AO.mult,
                  )
                  nc.vector.reciprocal(t2[:], t2[:])
                  nc.vector.tensor_mul(kap[:], sq_sb[:], t2[:])
                  nc.tensor.matmul(
                      kb_ps[:].rearrange("p a b c -> p (a b c)"),
                      lhsT=ones_row[:],
                      rhs=kap[:],
                      start=True,
                      stop=True,
                  )

                  # kappa broadcast leaves PSUM via ACT for the same reason
                  kb_sb = sm.tile([128, CH, N_DIGIT, BL], F32, tag="kb_sb")
                  nc.scalar.copy(out=kb_sb[:], in_=kb_ps[:])
                  if not last:
                      # ---- A-pass: a[i,(c,j)] = sum_q x[i,q] gx[j,c,q];
                      #      bl += kappa * a ----
                      for b in range(BL):
                          for k in range(NCHUNK):
                              a_ps = ps_a.tile([128, CH, N_DIGIT], F32, tag="a")
                              for c in range(CH):
                                  nc.tensor.matmul(
                                      a_ps[:, c, :],
                                      lhsT=xt[:, b, c, k, :],
                                      rhs=gx_bf[:, c, :, b],
                                      start=True,
                                      stop=True,
                                  )
                              # a leaves PSUM via ACT so the bank is released
                              # by the short queue, not the deep DVE queue
                              a_sb = sm.tile([128, CH, N_DIGIT], F32, tag="a_sb", bufs=2)
                              nc.scalar.copy(out=a_sb[:], in_=a_ps[:])
                              if t == 0:
                                  nc.vector.tensor_mul(
                                      bl_t[b][:, k], a_sb[:], kb_sb[:, :, :, b]
                                  )
                              else:
                                  a2 = sm.tile([128, CH, N_DIGIT], F32, tag="a2")
                                  nc.vector.tensor_mul(a2[:], a_sb[:], kb_sb[:, :, :, b])
                                  nc.vector.tensor_add(bl_t[b][:, k], bl_t[b][:, k], a2[:])
                  else:
                      # ---- output: v = kappa*s; transpose [p,(c,j,b)] ->
                      #      [(c,j,b),p] and DMA ----
                      vf = sm.tile([128, CH, N_DIGIT, BL], F32, tag="vf")
                      nc.vector.tensor_mul(vf[:], gx_f[:], kb_sb[:])
                      vflat = vf[:].rearrange("p a b c -> p (a b c)")
                      out_ap = out_d[:].rearrange("b j c p -> c j b p")  # [4,32,2,128]
                      for half in range(2):
                          tr_ps = ps_a.tile([128, 128], F32, tag="a")
                          nc.tensor.transpose(
                              tr_ps[:], vflat[:, half * 128 : (half + 1) * 128], ident[:]
                          )
                          ob = sm.tile([128, 128], F32, tag=f"ob{half}")
                          nc.scalar.copy(out=ob[:], in_=tr_ps[:])
                          for cl in range(2):
                              nc.sync.dma_start(
                                  out=out_ap[half * 2 + cl],
                                  in_=ob[cl * 64 : (cl + 1) * 64, :],
                              )

            if bench_reps:
                U = 4  # bodies per For_i iteration (amortizes the loop's
                       # all-engine barrier + semaphore reset)
                assert bench_reps % U == 0, f"bench_reps must be divisible by {U}"
                if bench_mode == "unroll":  # python-unrolled, for TimelineSim
                    load_x_g(0)
                    for r in range(bench_reps):
                        trace_body(r % 2, prefetch=True)
                elif bench_mode == "nodma":
                    load_x_g(0)
                    load_x_g(1)
                    load_wt()
                    with tc.For_i(0, bench_reps // U, 1):
                        for r in range(U):
                            trace_body(r % 2, loads=False)
                elif bench_mode == "dmaonly":
                    load_x_g(0)
                    with tc.For_i(0, bench_reps // U, 1):
                        for r in range(U):
                            trace_body(r % 2, prefetch=True, compute=False)
                else:
                    load_x_g(0)
                    with tc.For_i(0, bench_reps // U, 1):
                        for r in range(U):
                            trace_body(r % 2, prefetch=True)
            else:
                trace_body(0)
    return nc


def _np_dt(dt):
    if dt == BF16:
        import ml_dtypes

        return ml_dtypes.bfloat16
    return np.float32


def _host_prep(x: np.ndarray, w: np.ndarray):
    """Host-side layout prep shared by all cores (w-derived) and per-core (x)."""
    x = np.ascontiguousarray(x, dtype=np.float32)
    w = np.ascontiguousarray(w, dtype=np.float32)
    ndx, ndw = _np_dt(DT_X), _np_dt(DT_W)
    # G[j,c,q,r] = sum_p w[j,c,p,q] w[j,c,p,r]
    wf = np.ascontiguousarray(w.transpose(1, 0, 2, 3)).reshape(NJC, D, D)  # jc = c*32+j
    G = np.matmul(wf.transpose(0, 2, 1), wf)                 # [jc, q, r]
    g_h = np.ascontiguousarray(G.transpose(1, 0, 2)).astype(ndw)    # [q, jc, r]
    wt_h = np.ascontiguousarray(wf.transpose(2, 0, 1)).astype(ndw)  # [q, jc, p]
    # x[b,i,c,q] with i = k*128 + r  ->  xk [r, b, c, k, q], xt [q, b, c, k, r]
    xr = x.reshape(B, NCHUNK, 128, CH, D)
    xk_h = np.ascontiguousarray(xr.transpose(2, 0, 3, 1, 4)).astype(ndx)  # [r, b, c, k, q]
    xt_h = np.ascontiguousarray(xr.transpose(4, 0, 3, 1, 2)).astype(ndx)  # [q, b, c, k, r]
    return xk_h, xt_h, g_h, wt_h


def _in_maps(x: np.ndarray, w: np.ndarray):
    xk_h, xt_h, g_h, wt_h = _host_prep(x, w)
    in_maps = []
    for core in range(N_CORES):
        in_maps.append(
            {
                "xk": xk_h[:, core * BL : (core + 1) * BL],
                "xt": xt_h[:, core * BL : (core + 1) * BL],
                "g": g_h,
                "wt": wt_h,
            }
        )
    return in_maps


def _run(x: np.ndarray, w: np.ndarray, **spmd_kwargs):
    in_maps = _in_maps(x, w)
    nc = build_nc()
    nc.finalize()
    res = run_bass_kernel_spmd(nc, in_maps, list(range(N_CORES)), **spmd_kwargs)
    out = np.concatenate([res.results[c]["out"] for c in range(N_CORES)], axis=0)
    return out.astype(np.float32), res


def kernel(x: np.ndarray, w: np.ndarray) -> np.ndarray:
    out, _ = _run(x, w)
    return out


# revision 30
# speedup vs baseline: 1.1902x; 1.1902x over previous
"""Trainium2 Bass kernel for CapsNet dynamic routing (nn_Model_16492674417055).

Reference computation:
    u_hat[b,i,j,c,p] = sum_q w[j,c,p,q] x[b,i,c,q]
    3 routing iterations of: c = softmax_j(b); s = sum_i c*u_hat;
    v = squash(s); a = <u_hat, v>; b += a. Output v of last iteration.

Key algebraic factorization (exact in real arithmetic): u_hat never needs to
be materialized (it is 1 GiB).  With xc[b,j,c,:] = sum_i c[b,i,j,c] x[b,i,c,:]:
    s  = W @ xc
    a  = <x_i, W^T v>  and  W^T v = kappa * (W^T W) xc = kappa * G xc,
where kappa is the squash scale, computable from |s|^2 = <xc, G xc>.
So iterations 1..2 need only G = W^T W (host-precomputed), and the final
iteration needs one true W application for the output direction.

Sharding: data-parallel over batch B=16 across 8 cores (2 batches/core);
G / wT are replicated (loaded per core); routing state stays core-local.

Design notes (v3):
- Matmul operands in bf16 (DT_X for x/c/gx, DT_W for G/wt/xc): fp32 matmuls
  cost 4 cycles/row and serialize the weight load; bf16 runs 1 cycle/row with
  fast-weight-load LDWEIGHTS that overlap the previous matmul.  All routing
  state stays fp32 (PSUM accumulation, logits, softmax, squash scalars).
  Measured rel err 1.6e-2 vs the 2e-2 gate (fp32 everywhere gives 3e-6).
- kappa is folded into the logit update (bl += kappa * <x_i, gx>) instead of
  scaling vt = kappa*gx before the A-pass, so the 64 A-matmuls start right
  after the W-pass while the squash scalar chain runs in parallel.
- xk/g/xt are double-buffered: in the benchmark loop each body prefetches the
  next body's buffers at its start, so input DMA overlaps the previous body's
  tail instead of stalling each rep's head.  wt is single-buffered (read late
  at t=2, loaded early per body).
- XC accumulates all 4 channels of one batch into a single PSUM bank
  (32 matmuls), one PSUM->SBUF cast-copy per batch.
- tc.For_i runs an all-engine barrier + semaphore reset every loop iteration,
  which drains the cross-body pipeline; the bench loop therefore unrolls U=4
  bodies per For_i iteration (113us/rep at U=2 -> ~76-91us at U=4 under the
  paired-slope protocol; U=8 was worse).
- PSUM tiles must be released by SHORT-queue engines: every PSUM consumer
  whose result feeds DVE reads an ACT-made SBUF copy (gx_bf, sq_sb, kb_sb).
  A DVE read of PSUM parks the bank behind the deep DVE queue and stalls the
  PE matmuls that want the bank two iterations later (measured 2x slowdown).
- Roofline context: ~704 LDW+MM pairs x ~55-80ns ~= 39-56us on PE and 12 MiB
  of input DMA ~= 35-55us per core; measured per-rep sits just above both.
"""

import numpy as np

import concourse.bass as bass
import concourse.tile as tile
from concourse import bacc
from concourse import mybir
from concourse.alu_op_type import AluOpType as AO
from concourse.bass import MemorySpace
from concourse.bass_utils import run_bass_kernel_spmd
from concourse.masks import make_identity

F32 = mybir.dt.float32
BF16 = mybir.dt.bfloat16
AXX = mybir.AxisListType.X
AF = mybir.ActivationFunctionType

N_CORES = 8
B, N_PRE, N_DIGIT, CH, D = 16, 1024, 32, 4, 128
BL = B // N_CORES          # batches per core (2)
NCHUNK = N_PRE // 128      # i-chunks (8)
NJC = N_DIGIT * CH         # 128 (j,c) pairs
EPS = 1e-7
N_ITERS = 3

# matmul operand dtypes (state is always fp32); matmul needs matching operand
# dtypes: DT_X covers xk/xt/softmax-c/gx(A-pass rhs), DT_W covers G/wT/xc.
DT_X = BF16
DT_W = BF16


class _Bacc(bacc.Bacc):
    """Bacc whose ACT-table chooser only sees natural_log_exp_and_others, so
    alternating Exp (softmax) / Ln+Exp (squash sqrt) stay on ONE table set
    (one LoadActFuncSet instead of one per switch)."""

    def insert_act_table_loads(self):
        from concourse.hw_specs import get_activation_tables

        has_activation = any(
            isinstance(i, mybir.InstActivation)
            for b in self.main_func.blocks
            for i in b.instructions
        )
        if not has_activation:
            return
        tables = [
            (n, fns if n == "natural_log_exp_and_others" else set())
            for n, fns in get_activation_tables(self.m.arch).items()
        ]
        bacc._bass_rust.insert_act_table_loads(self, tables)


def build_nc(bench_reps: int = 0, bench_mode: str = "full") -> bass.Bass:
    """bench_reps>0 wraps the kernel body (input DMAs included) in a For_i
    loop (two ping-pong bodies per iteration) inside one NEFF, for wall-clock
    timing that amortizes the multi-ms axon dispatch floor."""
    nc = _Bacc()

    # Per-core DRAM inputs, host pre-laid-out so every load is a straight
    # [128, N] partition-major copy.
    xk_d = nc.declare_dram_parameter("xk", [128, BL, CH, NCHUNK, 128], DT_X, isOutput=False)  # [i128, b, c, k, q]
    xt_d = nc.declare_dram_parameter("xt", [128, BL, CH, NCHUNK, 128], DT_X, isOutput=False)  # [q, b, c, k, i128]
    g_d = nc.declare_dram_parameter("g", [128, NJC, 128], DT_W, isOutput=False)               # [q, (c j), q']
    wt_d = nc.declare_dram_parameter("wt", [128, NJC, 128], DT_W, isOutput=False)             # [q, (c j), p]
    out_d = nc.declare_dram_parameter("out", [BL, N_DIGIT, CH, D], F32, isOutput=True)

    with tile.TileContext(nc) as tc:
        with (
            tc.tile_pool(name="big", bufs=1) as big,
            tc.tile_pool(name="sm", bufs=2) as sm,
            tc.tile_pool(name="ps_xc", bufs=2, space=MemorySpace.PSUM) as ps_xc,
            tc.tile_pool(name="ps_gx", bufs=2, space=MemorySpace.PSUM) as ps_gx,
            tc.tile_pool(name="ps_a", bufs=2, space=MemorySpace.PSUM) as ps_a,
            tc.tile_pool(name="ps_sq", bufs=1, space=MemorySpace.PSUM) as ps_sq,
            tc.tile_pool(name="ps_kb", bufs=1, space=MemorySpace.PSUM) as ps_kb,
        ):
            # ---- static tiles (xk/g/xt ping-pong buffered) ----
            xk2 = [big.tile([128, BL, CH, NCHUNK, 128], DT_X, tag=f"xk{p}", name=f"xk{p}") for p in range(2)]
            xt2 = [big.tile([128, BL, CH, NCHUNK, 128], DT_X, tag=f"xt{p}", name=f"xt{p}") for p in range(2)]
            g2 = [big.tile([128, NJC, 128], DT_W, tag=f"g{p}", name=f"g{p}") for p in range(2)]
            wt = big.tile([128, NJC, 128], DT_W, tag="wt")

            c_unif = big.tile([128, N_DIGIT], DT_X, tag="c_unif")
            nc.vector.memset(c_unif, 1.0 / N_DIGIT)
            ones_col = big.tile([128, 1], F32, tag="ones_col")
            nc.vector.memset(ones_col, 1.0)
            ones_row = big.tile([1, 128], F32, tag="ones_row")
            nc.vector.memset(ones_row, 1.0)
            ident = big.tile([128, 128], F32, tag="ident")
            make_identity(nc, ident[:])
            eps_t = big.tile([1, 1], F32, tag="eps_t")
            nc.vector.memset(eps_t, EPS)

            # routing logits per local batch: [i%128, k, c, j]
            bl_t = [big.tile([128, NCHUNK, CH, N_DIGIT], F32, tag=f"bl{b}", name=f"bl{b}") for b in range(BL)]

            def load_x_g(p):
                nc.sync.dma_start(out=xk2[p][:], in_=xk_d[:])
                nc.scalar.dma_start(out=g2[p][:], in_=g_d[:])
                nc.scalar.dma_start(out=xt2[p][:], in_=xt_d[:])

            def load_wt():
                nc.sync.dma_start(out=wt[:], in_=wt_d[:])

            def trace_body(p, prefetch=False, loads=True, compute=True):
              # prefetch=True (pipelined bench loop): load NEXT parity's
              # x/g buffers + this body's wt.  Otherwise self-load parity p.
              if prefetch:
                  load_x_g(1 - p)
                  load_wt()
              elif loads:
                  load_x_g(p)
                  load_wt()
              if not compute:
                  return
              xk, xt, gt = xk2[p], xt2[p], g2[p]
              for t in range(N_ITERS):
                  last = t == N_ITERS - 1

                  # ---- softmax over j (t=0: uniform, skip) ----
                  cbt = []
                  if t > 0:
                      for b in range(BL):
                          # b=0 on DVE, b=1 on GpSimd (parallel chains)
                          ve = nc.vector if b == 0 else nc.gpsimd
                          blv = bl_t[b][:]
                          mx = sm.tile([128, NCHUNK, CH], F32, tag=f"mx{b}")
                          nc.vector.reduce_max(out=mx[:], in_=blv, axis=AXX, negate=True)
                          eb = sm.tile([128, NCHUNK, CH, N_DIGIT], DT_X, tag=f"e{b}")
                          ve.tensor_add(eb[:], blv, mx[:].to_broadcast(eb.shape))
                          nc.scalar.activation(eb[:], eb[:], AF.Exp)
                          sb = sm.tile([128, NCHUNK, CH], F32, tag=f"sum{b}")
                          nc.vector.reduce_sum(out=sb[:], in_=eb[:], axis=AXX)
                          nc.vector.reciprocal(sb[:], sb[:])
                          # normalize in place: eb becomes the c coefficients
                          ve.tensor_mul(eb[:], eb[:], sb[:].to_broadcast(eb.shape))
                          cbt.append(eb)

                  # ---- XC: xcT[q, (c,j,b)]; 4 channels share one PSUM bank ----
                  xc_sb = sm.tile([128, CH, N_DIGIT, BL], DT_W, tag="xc", bufs=4)
                  for b in range(BL):
                      xc_ps = ps_xc.tile([128, CH, N_DIGIT], F32, tag="xc_ps")
                      for c in range(CH):
                          for k in range(NCHUNK):
                              rhs = cbt[b][:, k, c, :] if t > 0 else c_unif[:]
                              nc.tensor.matmul(
                                  xc_ps[:, c, :],
                                  lhsT=xk[:, b, c, k, :],
                                  rhs=rhs,
                                  start=(k == 0),
                                  stop=(k == NCHUNK - 1),
                              )
                      nc.vector.tensor_copy(xc_sb[:, :, :, b], xc_ps[:])

                  # ---- W-pass: gxT = G @ xc (t<2)  /  sT = W @ xc (t=2) ----
                  gx_ps = ps_gx.tile([128, CH, N_DIGIT, BL], F32, tag="gx")
                  wsrc = wt if last else gt
                  for jc in range(NJC):
                      c, j = divmod(jc, N_DIGIT)
                      nc.tensor.matmul(
                          gx_ps[:, c, j, :],
                          lhsT=wsrc[:, jc, :],
                          rhs=xc_sb[:, c, j, :],
                          start=True,
                          stop=True,
                      )

                  # ---- squash scale kappa (batched over c,j,b) ----
                  # |s|^2 = <xc, G xc> (t<2) or <s, s> (t=2);
                  # kappa = sq/((1+sq)*sqrt(sq+eps)); sqrt = exp(0.5*ln) so only
                  # the natural_log_exp ACT table set is used.
                  xg = sm.tile([128, CH, N_DIGIT, BL], F32, tag="xg")
                  sq_ps = ps_sq.tile([1, CH * N_DIGIT * BL], F32, tag="sq")
                  kb_ps = ps_kb.tile([128, CH, N_DIGIT, BL], F32, tag="kb")
                  t1 = sm.tile([1, CH * N_DIGIT * BL], F32, tag="t1")
                  t2 = sm.tile([1, CH * N_DIGIT * BL], F32, tag="t2")
                  kap = sm.tile([1, CH * N_DIGIT * BL], F32, tag="kap")
                  if not last:
                      # bf16 copy of gx feeds the A-pass immediately; kappa is
                      # applied later, during the logit update.  xg reads the
                      # SBUF copy, not gx_ps: the PSUM bank must be released by
                      # the short ACT queue, not the deep DVE queue, or the
                      # W-pass two iterations later stalls on the rotation.
                      gx_bf = sm.tile([128, CH, N_DIGIT, BL], DT_X, tag="gx_bf", bufs=4)
                      nc.scalar.copy(out=gx_bf[:], in_=gx_ps[:])
                      nc.vector.tensor_mul(xg[:], xc_sb[:], gx_bf[:])
                  else:
                      gx_f = sm.tile([128, CH, N_DIGIT, BL], F32, tag="gx_f")
                      nc.scalar.copy(out=gx_f[:], in_=gx_ps[:])
                      nc.vector.tensor_mul(xg[:], gx_f[:], gx_f[:])
                  nc.tensor.matmul(
                      sq_ps[:],
                      lhsT=ones_col[:],
                      rhs=xg[:].rearrange("p a b c -> p (a b c)"),
                      start=True,
                      stop=True,
                  )
                  # sq leaves PSUM via the short ACT queue (releases the bank
                  # for the next iteration's sq matmul); DVE reads the copy.
                  sq_sb = sm.tile([1, CH * N_DIGIT * BL], F32, tag="sq_sb")
                  nc.scalar.copy(out=sq_sb[:], in_=sq_ps[:])
                  nc.scalar.activation(t1[:], sq_sb[:], AF.Ln, bias=eps_t[:])
                  nc.scalar.activation(t1[:], t1[:], AF.Exp, scale=0.5)
                  nc.vector.scalar_tensor_tensor(
                      out=t2[:], in0=sq_sb[:], scalar=1.0,
                      in1=t1[:], op0=AO.add, op1=AO.mult,
                  )
                  nc.vector.reciprocal(t2[:], t2[:])
                  nc.vector.tensor_mul(kap[:], sq_sb[:], t2[:])
                  nc.tensor.matmul(
                      kb_ps[:].rearrange("p a b c -> p (a b c)"),
                      lhsT=ones_row[:],
                      rhs=kap[:],
                      start=True,
                      stop=True,
                  )

                  # kappa broadcast leaves PSUM via ACT for the same reason
                  kb_sb = sm.tile([128, CH, N_DIGIT, BL], F32, tag="kb_sb")
                  nc.scalar.copy(out=kb_sb[:], in_=kb_ps[:])
                  if not last:
                      # ---- A-pass: a[i,(c,j)] = sum_q x[i,q] gx[j,c,q];
                      #      bl += kappa * a ----
                      for b in range(BL):
                          for k in range(NCHUNK):
                              a_ps = ps_a.tile([128, CH, N_DIGIT], F32, tag="a")
                              for c in range(CH):
                                  nc.tensor.matmul(
                                      a_ps[:, c, :],
                                      lhsT=xt[:, b, c, k, :],
                                      rhs=gx_bf[:, c, :, b],
                                      start=True,
                                      stop=True,
                                  )
                              if t == 0:
                                  nc.vector.tensor_mul(
                                      bl_t[b][:, k], a_ps[:], kb_sb[:, :, :, b]
                                  )
                              else:
                                  a2 = sm.tile([128, CH, N_DIGIT], F32, tag="a2", bufs=3)
                                  nc.vector.tensor_mul(a2[:], a_ps[:], kb_sb[:, :, :, b])
                                  nc.vector.tensor_add(bl_t[b][:, k], bl_t[b][:, k], a2[:])
                  else:
                      # ---- output: v = kappa*s; transpose [p,(c,j,b)] ->
                      #      [(c,j,b),p] and DMA ----
                      vf = sm.tile([128, CH, N_DIGIT, BL], F32, tag="vf")
                      nc.vector.tensor_mul(vf[:], gx_f[:], kb_sb[:])
                      vflat = vf[:].rearrange("p a b c -> p (a b c)")
                      out_ap = out_d[:].rearrange("b j c p -> c j b p")  # [4,32,2,128]
                      for half in range(2):
                          tr_ps = ps_a.tile([128, 128], F32, tag="a")
                          nc.tensor.transpose(
                              tr_ps[:], vflat[:, half * 128 : (half + 1) * 128], ident[:]
                          )
                          ob = sm.tile([128, 128], F32, tag=f"ob{half}")
                          nc.scalar.copy(out=ob[:], in_=tr_ps[:])
                          for cl in range(2):
                              nc.sync.dma_start(
                                  out=out_ap[half * 2 + cl],
                                  in_=ob[cl * 64 : (cl + 1) * 64, :],
                              )

            if bench_reps:
                U = 4  # bodies per For_i iteration (amortizes the loop's
                       # all-engine barrier + semaphore reset)
                assert bench_reps % U == 0, f"bench_reps must be divisible by {U}"
                if bench_mode == "unroll":  # python-unrolled, for TimelineSim
                    load_x_g(0)
                    for r in range(bench_reps):
                        trace_body(r % 2, prefetch=True)
                elif bench_mode == "nodma":
                    load_x_g(0)
                    load_x_g(1)
                    load_wt()
                    with tc.For_i(0, bench_reps // U, 1):
                        for r in range(U):
                            trace_body(r % 2, loads=False)
                elif bench_mode == "dmaonly":
                    load_x_g(0)
                    with tc.For_i(0, bench_reps // U, 1):
                        for r in range(U):
                            trace_body(r % 2, prefetch=True, compute=False)
                else:
                    load_x_g(0)
                    with tc.For_i(0, bench_reps // U, 1):
                        for r in range(U):
                            trace_body(r % 2, prefetch=True)
            else:
                trace_body(0)
    return nc


def _np_dt(dt):
    if dt == BF16:
        import ml_dtypes

        return ml_dtypes.bfloat16
    return np.float32


def _host_prep(x: np.ndarray, w: np.ndarray):
    """Host-side layout prep shared by all cores (w-derived) and per-core (x)."""
    x = np.ascontiguousarray(x, dtype=np.float32)
    w = np.ascontiguousarray(w, dtype=np.float32)
    ndx, ndw = _np_dt(DT_X), _np_dt(DT_W)
    # G[j,c,q,r] = sum_p w[j,c,p,q] w[j,c,p,r]
    wf = np.ascontiguousarray(w.transpose(1, 0, 2, 3)).reshape(NJC, D, D)  # jc = c*32+j
    G = np.matmul(wf.transpose(0, 2, 1), wf)                 # [jc, q, r]
    g_h = np.ascontiguousarray(G.transpose(1, 0, 2)).astype(ndw)    # [q, jc, r]
    wt_h = np.ascontiguousarray(wf.transpose(2, 0, 1)).astype(ndw)  # [q, jc, p]
    # x[b,i,c,q] with i = k*128 + r  ->  xk [r, b, c, k, q], xt [q, b, c, k, r]
    xr = x.reshape(B, NCHUNK, 128, CH, D)
    xk_h = np.ascontiguousarray(xr.transpose(2, 0, 3, 1, 4)).astype(ndx)  # [r, b, c, k, q]
    xt_h = np.ascontiguousarray(xr.transpose(4, 0, 3, 1, 2)).astype(ndx)  # [q, b, c, k, r]
    return xk_h, xt_h, g_h, wt_h


def _in_maps(x: np.ndarray, w: np.ndarray):
    xk_h, xt_h, g_h, wt_h = _host_prep(x, w)
    in_maps = []
    for core in range(N_CORES):
        in_maps.append(
            {
                "xk": xk_h[:, core * BL : (core + 1) * BL],
                "xt": xt_h[:, core * BL : (core + 1) * BL],
                "g": g_h,
                "wt": wt_h,
            }
        )
    return in_maps


def _run(x: np.ndarray, w: np.ndarray, **spmd_kwargs):
    in_maps = _in_maps(x, w)
    nc = build_nc()
    nc.finalize()
    res = run_bass_kernel_spmd(nc, in_maps, list(range(N_CORES)), **spmd_kwargs)
    out = np.concatenate([res.results[c]["out"] for c in range(N_CORES)], axis=0)
    return out.astype(np.float32), res


def kernel(x: np.ndarray, w: np.ndarray) -> np.ndarray:
    out, _ = _run(x, w)
    return out
